# revision 8
# baseline (speedup 1.0000x reference)
"""Trainium2 Bass kernel for nn_BitResidualBlock (dense_cnn).

Reference computation (per batch element, C=512 channels, T=4096):
    for d in (1, 3, 5):
        h = bitconv1d(x, w1, b1, dilation=d)     # ternary-quantized weights
        h = snake_beta(h, alpha, beta)           # x + sin(a*x)^2 / (b+eps)
        h = bitconv1d(h, w2, b2, dilation=1)
        x = x + h

Strategy:
  - Data-parallel over batch: 8 batch elements -> 8 NeuronCores.
  - conv1 (dilations 1,3,5): direct form, 12 accumulating 128x128x512
    matmuls per psum tile; bf16 operands, f32 PSUM.
  - conv2 (dilation 1): hybrid Winograd F(2,3). Output pairs
    y[2j]   = m0+m1+m2
    y[2j+1] = m1-m2-m3,   m_t = (G w)_t @ (B^T d)_t
    grouped into three PSUM chains: A = m0+m2, B = m1, D = -m2-m3
    (negated weights shipped from host), so y0 = A+B, y1 = B+D needs
    only 2 DVE combines while streaming 5/6 of direct's columns
    (20 vs 24 matmuls per 1024 outputs).
  - snake output h is stored DEINTERLEAVED (he=h[0::2], ho=h[1::2]) so
    the Winograd input transforms are contiguous shifted-view adds:
      dh0 = ho[j-1]-ho[j], dh1 = he[j]+ho[j],
      dh2 = ho[j]-he[j],   dh3 = he[j]-he[j+1]
    (run on GpSimd/DVE; GpSimd has no PSUM port and dislikes strides,
    so it gets only contiguous SBUF->SBUF work).
  - Residual stream kept in bf16 (ping-pong xbA/xbB); emulated rel_l2
    0.0069 vs reference (budget 2e-2).
  - snake: r=(a/pi)z on ScalarE from z; i32 trunc round-trip for range
    reduction (HW rejects ALU mod); sin on ScalarE; u^2 in bf16.
"""

import numpy as np
import ml_dtypes

import concourse.bass as bass
import concourse.mybir as mybir
import concourse.tile as tile
from concourse.vector_clock import ScopedClock
from concourse.bass_utils import run_bass_kernel_spmd

AF = mybir.ActivationFunctionType
ALU = mybir.AluOpType
F32 = mybir.dt.float32
I32 = mybir.dt.int32
BF16 = mybir.dt.bfloat16

B, C, T, K = 8, 512, 4096, 3
DILATIONS = (1, 3, 5)
EPS_Q = 1e-5
EPS_SNAKE = 1e-9

P = 128          # partitions
NCH = C // P     # 4 channel chunks
TT = 512         # time-tile (one PSUM bank of f32)
NT = T // TT     # 8 time tiles
NP = NT // 2     # 4 pair-groups per chunk
TT2 = 2 * TT     # pair width
PAD = 8          # zero pad each side of bf16 activation tiles
TPW = T + 2 * PAD
NPARAM = 18      # 6 param columns per block x 3 blocks

J = T // 2       # winograd output pairs per row (2048)
JW = 512         # winograd j-slab
NG = J // JW     # 4 slabs
HPAD = 2         # pad cols each side of he/ho
HW_ = J + 2 * HPAD
NS1 = 3          # weight slots for direct conv1
NS2 = 5          # weight slots for winograd conv2: w^0,w^1,w^2,-w^2,-w^3

TRACE = False
LAST_EXEC_NS = None
LAST_RESULT = None


class SplitDrainTileContext(tile.TileContext):
    """TileContext whose tail drain splits its sem waits across
    single-wait instructions (walrus rejects multi-wait Drains)."""

    def _drain_and_barrier(self, tick_clock, wait_clock):
        collector = self.nc.sync.nop(nofuse=True)
        wait_clock.add_sem_waits(
            collector.ins, ScopedClock({None: tick_clock.global_clock})
        )
        si = collector.ins.sync_info
        waits = list(si.on_wait) if si is not None else []
        if len(waits) > 1:
            collector.ins.sync_info = mybir.SyncInfo(
                on_wait=waits[:1], on_update=list(si.on_update)
            )
            for w in waits[1:]:
                extra = self.nc.sync.nop(nofuse=True)
                extra.ins.sync_info = mybir.SyncInfo(on_wait=[w], on_update=[])
        self.nc.sync.drain()
        self.nc.all_engine_barrier()
        assert self.sems is not None
        popped = self.nc._tile_sem_poison_stack.pop()
        assert popped is self._sem_poison
        self.nc.clear_and_free_semaphores(list(self.sems.allocated().values()))
        self.nc.all_engine_barrier()


def _split_sync_waits(nc, maxw=1):
    """Move excess sync waits onto single-wait EventSemaphore
    instructions inserted just before the owner on the same engine."""
    for bb in nc.main_func.blocks:
        out = []
        changed = False
        for ins in bb.instructions:
            si = getattr(ins, "sync_info", None)
            if si is not None and len(si.on_wait) > maxw:
                waits = list(si.on_wait)
                extra, keep = waits[:-maxw], waits[-maxw:]
                for w in extra:
                    ev = mybir.InstEventSemaphore(
                        name=nc.get_next_instruction_name(), ins=[], outs=[])
                    ev.engine = ins.engine
                    ev.sync_info = mybir.SyncInfo(on_wait=[w], on_update=[])
                    nc.register_instruction(ev, overwrite=True)
                    out.append(ev)
                ins.sync_info = mybir.SyncInfo(
                    on_wait=keep, on_update=list(si.on_update))
                changed = True
            out.append(ins)
        if changed:
            bb.instructions = out


def build_nc():
    nc = bass.Bass(target_bir_lowering=False)
    xb16_d = nc.dram_tensor("xb16", [C, T], BF16, kind="ExternalInput")
    wt_d = nc.dram_tensor("wt", [3, 2, NCH, P, NS2 * NCH * P], BF16,
                          kind="ExternalInput")
    pp_d = nc.dram_tensor("pp", [NCH, P, NPARAM], F32, kind="ExternalInput")
    y_d = nc.dram_tensor("y", [C, T], F32, kind="ExternalOutput")

    with SplitDrainTileContext(nc) as tc:
        with (
            tc.tile_pool(name="persist", bufs=1) as p1,
            tc.tile_pool(name="wts", bufs=1) as pw,
            tc.tile_pool(name="ep2", bufs=2) as p2,
            tc.tile_pool(name="epz", bufs=2) as pz,
            tc.tile_pool(name="dh", bufs=2) as pdh,
            tc.tile_pool(name="ps", bufs=8, space="PSUM") as pps,
        ):
            xb = [[p1.tile([P, TPW], BF16, tag=f"x{s}{c}", name=f"x{s}{c}")
                   for c in range(NCH)] for s in range(2)]
            he = [p1.tile([P, HW_], BF16, tag=f"he{c}", name=f"he{c}")
                  for c in range(NCH)]
            ho = [p1.tile([P, HW_], BF16, tag=f"ho{c}", name=f"ho{c}")
                  for c in range(NCH)]
            pt = [p1.tile([P, NPARAM], F32, tag=f"pt{c}", name=f"pt{c}")
                  for c in range(NCH)]

            for s in range(2):
                for c in range(NCH):
                    nc.vector.memset(xb[s][c][:, 0:PAD], 0.0)
                    nc.vector.memset(xb[s][c][:, PAD + T:TPW], 0.0)
            for c in range(NCH):
                nc.vector.memset(he[c][:, 0:HPAD], 0.0)
                nc.vector.memset(he[c][:, HPAD + J:HW_], 0.0)
                nc.vector.memset(ho[c][:, 0:HPAD], 0.0)
                nc.vector.memset(ho[c][:, HPAD + J:HW_], 0.0)

            def alloc_w(i, conv):
                ns = NS1 if conv == 1 else NS2
                return [pw.tile([P, ns * NCH * P], BF16,
                                tag=f"w{conv}_{c}", name=f"w{conv}_{i}_{c}")
                        for c in range(NCH)]

            def load_weights(i):
                w1t, w2t = alloc_w(i, 1), alloc_w(i, 2)
                for c in range(NCH):
                    nc.sync.dma_start(out=w1t[c],
                                      in_=wt_d[i, 0, c][:, 0:NS1 * NCH * P])
                for c in range(NCH):
                    nc.sync.dma_start(out=w2t[c], in_=wt_d[i, 1, c])
                return w1t, w2t

            # Startup DMA order (single HWDGE FIFO is the critical path):
            # co=0 strip of w1 block0, xb tiles 0..2, params, rest of w1,
            # rest of xb, w2.
            w1t0 = alloc_w(0, 1)
            CW = K * P
            for c in range(NCH):
                nc.sync.dma_start(out=w1t0[c][:, 0:CW],
                                  in_=wt_d[0, 0, c][:, 0:CW])
            for jt in range(3):
                for c in range(NCH):
                    sl = slice(jt * TT, (jt + 1) * TT)
                    nc.sync.dma_start(
                        out=xb[0][c][:, PAD + jt * TT:PAD + (jt + 1) * TT],
                        in_=xb16_d[c * P:(c + 1) * P, sl])
            for c in range(NCH):
                nc.sync.dma_start(out=pt[c], in_=pp_d[c])
            for c in range(NCH):
                nc.sync.dma_start(out=w1t0[c][:, CW:NS1 * NCH * P],
                                  in_=wt_d[0, 0, c][:, CW:NS1 * NCH * P])
            for jt in range(3, NT):
                for c in range(NCH):
                    sl = slice(jt * TT, (jt + 1) * TT)
                    nc.sync.dma_start(
                        out=xb[0][c][:, PAD + jt * TT:PAD + (jt + 1) * TT],
                        in_=xb16_d[c * P:(c + 1) * P, sl])
            w2t0 = alloc_w(0, 2)
            for c in range(NCH):
                nc.sync.dma_start(out=w2t0[c], in_=wt_d[0, 1, c])
            wcur = (w1t0, w2t0)

            for i in range(3):
                d = DILATIONS[i]
                base = i * 6
                w1t, w2t = wcur
                if i < 2:
                    wnext = load_weights(i + 1)
                xcur = xb[i % 2]
                xnxt = xb[(i + 1) % 2]

                # ---- conv1 (direct, dilation d) + snake -> he/ho ----
                for co in range(NCH):
                    b1ap = pt[co][:, base + 0:base + 1]
                    s1ap = pt[co][:, base + 1:base + 2]
                    apap = pt[co][:, base + 2:base + 3]
                    ibap = pt[co][:, base + 3:base + 4]
                    for jp in range(NP):
                        col0 = PAD + jp * TT2
                        z2 = pz.tile([P, TT2], F32, tag="z2")
                        for h2 in range(2):
                            ps = pps.tile([P, TT], F32, tag="ps")
                            c0 = col0 + h2 * TT
                            n = 0
                            for ci in range(NCH):
                                for k in range(K):
                                    sh = (k - 1) * d
                                    nc.tensor.matmul(
                                        ps,
                                        w1t[ci][:, (co * K + k) * P:
                                                (co * K + k + 1) * P],
                                        xcur[ci][:, c0 + sh:c0 + sh + TT],
                                        start=(n == 0), stop=(n == 11),
                                    )
                                    n += 1
                            nc.scalar.activation(
                                z2[:, h2 * TT:(h2 + 1) * TT], ps,
                                AF.Identity, bias=b1ap, scale=s1ap)
                        # snake: r=(a/pi)z; dd=r-int(r); u=sin(pi dd);
                        # h = z + invb*u^2 (sin^2 is pi-periodic so the
                        # trunc-vs-round cast ambiguity is harmless)
                        r2 = p2.tile([P, TT2], F32, tag="r2")
                        nc.scalar.activation(r2, z2, AF.Identity, scale=apap)
                        ri = p2.tile([P, TT2], mybir.dt.int16, tag="ri")
                        nc.gpsimd.tensor_copy(ri, r2)
                        d2 = p2.tile([P, TT2], F32, tag="d2")
                        nc.vector.tensor_sub(d2, r2, ri)
                        u2 = p2.tile([P, TT2], BF16, tag="u2")
                        nc.scalar.activation(u2, d2, AF.Sin,
                                             scale=float(np.pi))
                        v2 = p2.tile([P, TT2], BF16, tag="v2")
                        nc.vector.tensor_mul(v2, u2, u2)
                        # deinterleave h into he/ho (strided reads,
                        # contiguous writes)
                        hc0 = HPAD + jp * TT
                        nc.vector.scalar_tensor_tensor(
                            he[co][:, hc0:hc0 + TT],
                            v2[:, 0:TT2:2], ibap, z2[:, 0:TT2:2],
                            ALU.mult, ALU.add)
                        nc.vector.scalar_tensor_tensor(
                            ho[co][:, hc0:hc0 + TT],
                            v2[:, 1:TT2:2], ibap, z2[:, 1:TT2:2],
                            ALU.mult, ALU.add)

                # ---- conv2 (winograd F(2,3), dilation 1) + residual ----
                b2aps = [pt[co][:, base + 4:base + 5] for co in range(NCH)]
                s2aps = [pt[co][:, base + 5:base + 6] for co in range(NCH)]
                for g in range(NG):
                    j0 = HPAD + g * JW
                    dh = [pdh.tile([P, 4 * JW], BF16, tag=f"dh{ci}",
                                   name=f"dh{i}_{g}_{ci}")
                          for ci in range(NCH)]
                    for ci in range(NCH):
                        # dh0 = ho[j-1]-ho[j]; dh1 = he[j]+ho[j]
                        # dh2 = ho[j]-he[j];   dh3 = he[j]-he[j+1]
                        nc.gpsimd.tensor_sub(
                            dh[ci][:, 0 * JW:1 * JW],
                            ho[ci][:, j0 - 1:j0 - 1 + JW],
                            ho[ci][:, j0:j0 + JW])
                        nc.gpsimd.tensor_add(
                            dh[ci][:, 1 * JW:2 * JW],
                            he[ci][:, j0:j0 + JW],
                            ho[ci][:, j0:j0 + JW])
                        nc.vector.tensor_sub(
                            dh[ci][:, 2 * JW:3 * JW],
                            ho[ci][:, j0:j0 + JW],
                            he[ci][:, j0:j0 + JW])
                        nc.gpsimd.tensor_sub(
                            dh[ci][:, 3 * JW:4 * JW],
                            he[ci][:, j0:j0 + JW],
                            he[ci][:, j0 + 1:j0 + 1 + JW])
                    for co in range(NCH):
                        # chains: A = m0+m2 (slots 0,2), B = m1 (slot 1),
                        #         D = -m2-m3 (slots 3,4)
                        psA = pps.tile([P, JW], F32, tag="ps")
                        psB = pps.tile([P, JW], F32, tag="ps")
                        psD = pps.tile([P, JW], F32, tag="ps")
                        for n, (pst, slot, st) in enumerate((
                                (psA, 0, 0), (psA, 2, 2),
                                (psB, 1, 1),
                                (psD, 3, 2), (psD, 4, 3))):
                            first = slot in (0, 1, 3)
                            last = slot in (2, 1, 4)
                            for ci in range(NCH):
                                nc.tensor.matmul(
                                    pst,
                                    w2t[ci][:, (co * NS2 + slot) * P:
                                            (co * NS2 + slot + 1) * P],
                                    dh[ci][:, st * JW:(st + 1) * JW],
                                    start=(first and ci == 0),
                                    stop=(last and ci == NCH - 1),
                                )
                        # walrus: a TensorTensor may read only ONE psum
                        # operand. Drain B via ScalarE with scale/bias
                        # folded (tB = s2*m1 + b2), then each output half
                        # is one stt reading a single psum bank:
                        #   t_even = s2*A + tB,  t_odd = s2*D + tB
                        # (tags shared with conv1-phase tiles, disjoint
                        # lifetime, to stay inside SBUF)
                        tBt = p2.tile([P, TT2], F32, tag="r2")
                        tB = tBt[:, 0:JW]
                        nc.scalar.activation(tB, psB, AF.Identity,
                                             bias=b2aps[co], scale=s2aps[co])
                        q01 = p2.tile([P, TT2], F32, tag="d2")
                        nc.vector.scalar_tensor_tensor(
                            q01[:, 0:JW], psA, s2aps[co], tB,
                            ALU.mult, ALU.add)
                        nc.vector.scalar_tensor_tensor(
                            q01[:, JW:2 * JW], psD, s2aps[co], tB,
                            ALU.mult, ALU.add)
                        col0 = PAD + g * TT2
                        if i < 2:
                            nc.vector.tensor_add(
                                xnxt[co][:, col0:col0 + TT2:2],
                                xcur[co][:, col0:col0 + TT2:2],
                                q01[:, 0:JW])
                            nc.vector.tensor_add(
                                xnxt[co][:, col0 + 1:col0 + TT2:2],
                                xcur[co][:, col0 + 1:col0 + TT2:2],
                                q01[:, JW:2 * JW])
                        else:
                            y2 = pz.tile([P, TT2], F32, tag="z2")
                            nc.vector.tensor_add(
                                y2[:, 0:TT2:2],
                                xcur[co][:, col0:col0 + TT2:2],
                                q01[:, 0:JW])
                            nc.vector.tensor_add(
                                y2[:, 1:TT2:2],
                                xcur[co][:, col0 + 1:col0 + TT2:2],
                                q01[:, JW:2 * JW])
                            nc.sync.dma_start(
                                out=y_d[co * P:(co + 1) * P,
                                        g * TT2:(g + 1) * TT2],
                                in_=y2)
                if i < 2:
                    wcur = wnext
    _split_sync_waits(nc)
    return nc


_NC = None


def _get_nc():
    global _NC
    if _NC is None:
        _NC = build_nc()
    return _NC


def _host_params(w1, b1, alpha, beta, w2, b2):
    """Ternarize weights and fold snake/scale params, matching the
    reference's jax-on-CPU float32 numerics."""
    import jax
    import jax.numpy as jnp

    cpu = jax.devices("cpu")[0]

    wt = np.zeros((3, 2, NCH, P, NS2 * NCH * P), dtype=ml_dtypes.bfloat16)
    pp = np.zeros((NCH, P, NPARAM), dtype=np.float32)
    pi = np.float32(np.pi)

    with jax.default_device(cpu):
        for i in range(3):
            svals = []
            for conv, w in ((0, w1[i]), (1, w2[i])):
                s = jnp.mean(jnp.abs(w))
                tern = jnp.clip(jnp.round(w / (s + EPS_Q)), -1.0, 1.0)
                svals.append(np.float32(s))
                tern = np.asarray(tern, dtype=np.float32)  # [co, ci, k]
                if conv == 0:
                    slots = tern  # direct: 3 tap slots
                    ns = NS1
                else:
                    # winograd slots: w^0, w^1, w^2, -w^2, -w^3
                    t0, t1, t2_ = tern[:, :, 0], tern[:, :, 1], tern[:, :, 2]
                    gw1 = (t0 + t1 + t2_) * np.float32(0.5)
                    gw2 = (t0 - t1 + t2_) * np.float32(0.5)
                    slots = np.stack([t0, gw1, gw2, -gw2, -t2_], axis=2)
                    ns = NS2
                # [co, ci, s] -> [cich, ci_in, coch, s, co_in]
                t5 = slots.reshape(NCH, P, NCH, P, ns).transpose(2, 3, 0, 4, 1)
                wt[i, conv, :, :, 0:ns * NCH * P] = t5.reshape(
                    NCH, P, ns * NCH * P).astype(ml_dtypes.bfloat16)
            s1, s2 = svals
            a = np.asarray(jnp.exp(alpha[i]), dtype=np.float32)
            bsn = np.asarray(jnp.exp(beta[i]), dtype=np.float32)
            invb = np.asarray(
                jnp.float32(1.0) / (jnp.asarray(bsn) + jnp.float32(EPS_SNAKE)),
                dtype=np.float32)
            base = i * 6
            pp[:, :, base + 0] = b1[i].reshape(NCH, P)
            pp[:, :, base + 1] = s1
            pp[:, :, base + 2] = (a / pi).reshape(NCH, P)
            pp[:, :, base + 3] = invb.reshape(NCH, P)
            pp[:, :, base + 4] = b2[i].reshape(NCH, P)
            pp[:, :, base + 5] = s2
    return wt, pp


def kernel(x, w1, b1, alpha, beta, w2, b2):
    global LAST_EXEC_NS
    x = np.asarray(x, dtype=np.float32)
    w1 = np.asarray(w1, dtype=np.float32)
    b1 = np.asarray(b1, dtype=np.float32)
    alpha = np.asarray(alpha, dtype=np.float32)
    beta = np.asarray(beta, dtype=np.float32)
    w2 = np.asarray(w2, dtype=np.float32)
    b2 = np.asarray(b2, dtype=np.float32)

    wt, pp = _host_params(w1, b1, alpha, beta, w2, b2)
    nc = _get_nc()

    in_maps = [
        {"xb16": x[b].astype(ml_dtypes.bfloat16), "wt": wt, "pp": pp}
        for b in range(B)
    ]
    res = run_bass_kernel_spmd(
        nc, in_maps, core_ids=list(range(B)), trace=TRACE)
    LAST_EXEC_NS = res.exec_time_ns
    global LAST_RESULT
    LAST_RESULT = res

    out = np.stack([res.results[b]["y"] for b in range(B)], axis=0)
    return out.astype(np.float32)


# revision 12
# speedup vs baseline: 1.1575x; 1.1575x over previous
"""Trainium2 Bass kernel for nn_BitResidualBlock (dense_cnn).

Reference computation (per batch element, C=512 channels, T=4096):
    for d in (1, 3, 5):
        h = bitconv1d(x, w1, b1, dilation=d)     # ternary-quantized weights
        h = snake_beta(h, alpha, beta)           # x + sin(a*x)^2 / (b+eps)
        h = bitconv1d(h, w2, b2, dilation=1)
        x = x + h

Strategy:
  - Data-parallel over batch: 8 batch elements -> 8 NeuronCores.
  - conv1 (dilations 1,3,5): direct form, 12 accumulating 128x128x512
    matmuls per psum tile; bf16 operands, f32 PSUM.
  - conv2 (dilation 1): hybrid Winograd F(2,3). Output pairs
    y[2j]   = m0+m1+m2
    y[2j+1] = m1-m2-m3,   m_t = (G w)_t @ (B^T d)_t
    grouped into three PSUM chains: A = m0+m2, B = m1, D = -m2-m3
    (negated weights shipped from host), so y0 = A+B, y1 = B+D needs
    only 2 DVE combines while streaming 5/6 of direct's columns
    (20 vs 24 matmuls per 1024 outputs).
  - snake output h is stored DEINTERLEAVED (he=h[0::2], ho=h[1::2]) so
    the Winograd input transforms are contiguous shifted-view adds:
      dh0 = ho[j-1]-ho[j], dh1 = he[j]+ho[j],
      dh2 = ho[j]-he[j],   dh3 = he[j]-he[j+1]
    (run on GpSimd/DVE; GpSimd has no PSUM port and dislikes strides,
    so it gets only contiguous SBUF->SBUF work).
  - Residual stream kept in bf16 (ping-pong xbA/xbB); emulated rel_l2
    0.0069 vs reference (budget 2e-2).
  - snake: r=(a/pi)z on ScalarE from z; i32 trunc round-trip for range
    reduction (HW rejects ALU mod); sin on ScalarE; u^2 in bf16.
"""

import numpy as np
import ml_dtypes

import concourse.bass as bass
import concourse.mybir as mybir
import concourse.tile as tile
from concourse.vector_clock import ScopedClock
from concourse.bass_utils import run_bass_kernel_spmd

AF = mybir.ActivationFunctionType
ALU = mybir.AluOpType
F32 = mybir.dt.float32
I32 = mybir.dt.int32
BF16 = mybir.dt.bfloat16

B, C, T, K = 8, 512, 4096, 3
DILATIONS = (1, 3, 5)
EPS_Q = 1e-5
EPS_SNAKE = 1e-9

P = 128          # partitions
NCH = C // P     # 4 channel chunks
TT = 512         # time-tile (one PSUM bank of f32)
NT = T // TT     # 8 time tiles
NP = NT // 2     # 4 pair-groups per chunk
TT2 = 2 * TT     # pair width
PAD = 8          # zero pad each side of bf16 activation tiles
TPW = T + 2 * PAD
NPARAM = 18      # 6 param columns per block x 3 blocks

J = T // 2       # winograd output pairs per row (2048)
JW = 512         # winograd j-slab
NG = J // JW     # 4 slabs
HPAD = 2         # pad cols each side of he/ho
HW_ = J + 2 * HPAD
NS1 = 3          # weight slots for direct conv1
NS2 = 5          # weight slots for winograd conv2: w^0,w^1,w^2,-w^2,-w^3

TRACE = False
LAST_EXEC_NS = None
LAST_RESULT = None


class SplitDrainTileContext(tile.TileContext):
    """TileContext whose tail drain splits its sem waits across
    single-wait instructions (walrus rejects multi-wait Drains)."""

    def _drain_and_barrier(self, tick_clock, wait_clock):
        collector = self.nc.sync.nop(nofuse=True)
        wait_clock.add_sem_waits(
            collector.ins, ScopedClock({None: tick_clock.global_clock})
        )
        si = collector.ins.sync_info
        waits = list(si.on_wait) if si is not None else []
        if len(waits) > 1:
            collector.ins.sync_info = mybir.SyncInfo(
                on_wait=waits[:1], on_update=list(si.on_update)
            )
            for w in waits[1:]:
                extra = self.nc.sync.nop(nofuse=True)
                extra.ins.sync_info = mybir.SyncInfo(on_wait=[w], on_update=[])
        self.nc.sync.drain()
        self.nc.all_engine_barrier()
        assert self.sems is not None
        popped = self.nc._tile_sem_poison_stack.pop()
        assert popped is self._sem_poison
        self.nc.clear_and_free_semaphores(list(self.sems.allocated().values()))
        self.nc.all_engine_barrier()


def _split_sync_waits(nc, maxw=1):
    """Move excess sync waits onto single-wait EventSemaphore
    instructions inserted just before the owner on the same engine."""
    for bb in nc.main_func.blocks:
        out = []
        changed = False
        for ins in bb.instructions:
            si = getattr(ins, "sync_info", None)
            if si is not None and len(si.on_wait) > maxw:
                waits = list(si.on_wait)
                extra, keep = waits[:-maxw], waits[-maxw:]
                for w in extra:
                    ev = mybir.InstEventSemaphore(
                        name=nc.get_next_instruction_name(), ins=[], outs=[])
                    ev.engine = ins.engine
                    ev.sync_info = mybir.SyncInfo(on_wait=[w], on_update=[])
                    nc.register_instruction(ev, overwrite=True)
                    out.append(ev)
                ins.sync_info = mybir.SyncInfo(
                    on_wait=keep, on_update=list(si.on_update))
                changed = True
            out.append(ins)
        if changed:
            bb.instructions = out


def build_nc():
    nc = bass.Bass(target_bir_lowering=False)
    xb16_d = nc.dram_tensor("xb16", [C, T], BF16, kind="ExternalInput")
    wt_d = nc.dram_tensor("wt", [3, 2, NCH, P, NS2 * NCH * P], BF16,
                          kind="ExternalInput")
    pp_d = nc.dram_tensor("pp", [NCH, P, NPARAM], F32, kind="ExternalInput")
    y_d = nc.dram_tensor("y", [C, T], F32, kind="ExternalOutput")

    with SplitDrainTileContext(nc) as tc:
        with (
            tc.tile_pool(name="persist", bufs=1) as p1,
            tc.tile_pool(name="wts", bufs=1) as pw,
            tc.tile_pool(name="ep2", bufs=2) as p2,
            tc.tile_pool(name="epz", bufs=2) as pz,
            tc.tile_pool(name="dh", bufs=2) as pdh,
            tc.tile_pool(name="ps", bufs=8, space="PSUM") as pps,
        ):
            xb = [[p1.tile([P, TPW], BF16, tag=f"x{s}{c}", name=f"x{s}{c}")
                   for c in range(NCH)] for s in range(2)]
            he = [p1.tile([P, HW_], BF16, tag=f"he{c}", name=f"he{c}")
                  for c in range(NCH)]
            ho = [p1.tile([P, HW_], BF16, tag=f"ho{c}", name=f"ho{c}")
                  for c in range(NCH)]
            pt = [p1.tile([P, NPARAM], F32, tag=f"pt{c}", name=f"pt{c}")
                  for c in range(NCH)]

            for s in range(2):
                for c in range(NCH):
                    nc.vector.memset(xb[s][c][:, 0:PAD], 0.0)
                    nc.vector.memset(xb[s][c][:, PAD + T:TPW], 0.0)
            for c in range(NCH):
                nc.vector.memset(he[c][:, 0:HPAD], 0.0)
                nc.vector.memset(he[c][:, HPAD + J:HW_], 0.0)
                nc.vector.memset(ho[c][:, 0:HPAD], 0.0)
                nc.vector.memset(ho[c][:, HPAD + J:HW_], 0.0)

            def alloc_w(i, conv):
                ns = NS1 if conv == 1 else NS2
                return [pw.tile([P, ns * NCH * P], BF16,
                                tag=f"w{conv}_{c}", name=f"w{conv}_{i}_{c}")
                        for c in range(NCH)]

            def load_weights(i):
                w1t, w2t = alloc_w(i, 1), alloc_w(i, 2)
                for c in range(NCH):
                    nc.sync.dma_start(out=w1t[c],
                                      in_=wt_d[i, 0, c][:, 0:NS1 * NCH * P])
                for c in range(NCH):
                    nc.sync.dma_start(out=w2t[c], in_=wt_d[i, 1, c])
                return w1t, w2t

            # Startup DMA order (single HWDGE FIFO is the critical path):
            # co=0 strip of w1 block0, xb tiles 0..2, params, rest of w1,
            # rest of xb, w2.
            w1t0 = alloc_w(0, 1)
            CW = K * P
            for c in range(NCH):
                nc.sync.dma_start(out=w1t0[c][:, 0:CW],
                                  in_=wt_d[0, 0, c][:, 0:CW])
            for jt in range(3):
                for c in range(NCH):
                    sl = slice(jt * TT, (jt + 1) * TT)
                    nc.sync.dma_start(
                        out=xb[0][c][:, PAD + jt * TT:PAD + (jt + 1) * TT],
                        in_=xb16_d[c * P:(c + 1) * P, sl])
            for c in range(NCH):
                nc.sync.dma_start(out=pt[c], in_=pp_d[c])
            for c in range(NCH):
                nc.sync.dma_start(out=w1t0[c][:, CW:NS1 * NCH * P],
                                  in_=wt_d[0, 0, c][:, CW:NS1 * NCH * P])
            for jt in range(3, NT):
                for c in range(NCH):
                    sl = slice(jt * TT, (jt + 1) * TT)
                    nc.sync.dma_start(
                        out=xb[0][c][:, PAD + jt * TT:PAD + (jt + 1) * TT],
                        in_=xb16_d[c * P:(c + 1) * P, sl])
            w2t0 = alloc_w(0, 2)
            for c in range(NCH):
                nc.sync.dma_start(out=w2t0[c], in_=wt_d[0, 1, c])
            wcur = (w1t0, w2t0)

            for i in range(3):
                d = DILATIONS[i]
                base = i * 6
                w1t, w2t = wcur
                if i < 2:
                    wnext = load_weights(i + 1)
                xcur = xb[i % 2]
                xnxt = xb[(i + 1) % 2]

                def emit_transforms(g):
                    # winograd input transforms for j-slab g, from the
                    # deinterleaved snake output (all contiguous views):
                    # dh0 = ho[j-1]-ho[j]; dh1 = he[j]+ho[j]
                    # dh2 = ho[j]-he[j];   dh3 = he[j]-he[j+1]
                    j0 = HPAD + g * JW
                    dh = [pdh.tile([P, 4 * JW], BF16, tag=f"dh{ci}",
                                   name=f"dh{i}_{g}_{ci}")
                          for ci in range(NCH)]
                    for ci in range(NCH):
                        nc.gpsimd.tensor_sub(
                            dh[ci][:, 0 * JW:1 * JW],
                            ho[ci][:, j0 - 1:j0 - 1 + JW],
                            ho[ci][:, j0:j0 + JW])
                        nc.vector.tensor_add(
                            dh[ci][:, 1 * JW:2 * JW],
                            he[ci][:, j0:j0 + JW],
                            ho[ci][:, j0:j0 + JW])
                        nc.vector.tensor_sub(
                            dh[ci][:, 2 * JW:3 * JW],
                            ho[ci][:, j0:j0 + JW],
                            he[ci][:, j0:j0 + JW])
                        nc.gpsimd.tensor_sub(
                            dh[ci][:, 3 * JW:4 * JW],
                            he[ci][:, j0:j0 + JW],
                            he[ci][:, j0 + 1:j0 + 1 + JW])
                    return dh

                # ---- conv1 (direct, dilation d) + snake -> he/ho ----
                # jp-outer so every chunk's early time-slabs finish first:
                # the winograd transforms for slab g only need conv1
                # output up to slab g+1, so they overlap conv1's matmuls
                # instead of gating the conv2 phase at the boundary.
                dh_ready = {}
                for jp in range(NP):
                    for co in range(NCH):
                        b1ap = pt[co][:, base + 0:base + 1]
                        s1ap = pt[co][:, base + 1:base + 2]
                        apap = pt[co][:, base + 2:base + 3]
                        ibap = pt[co][:, base + 3:base + 4]
                        col0 = PAD + jp * TT2
                        z2 = pz.tile([P, TT2], F32, tag="z2")
                        for h2 in range(2):
                            ps = pps.tile([P, TT], F32, tag="ps")
                            c0 = col0 + h2 * TT
                            n = 0
                            for ci in range(NCH):
                                for k in range(K):
                                    sh = (k - 1) * d
                                    nc.tensor.matmul(
                                        ps,
                                        w1t[ci][:, (co * K + k) * P:
                                                (co * K + k + 1) * P],
                                        xcur[ci][:, c0 + sh:c0 + sh + TT],
                                        start=(n == 0), stop=(n == 11),
                                    )
                                    n += 1
                            nc.scalar.activation(
                                z2[:, h2 * TT:(h2 + 1) * TT], ps,
                                AF.Identity, bias=b1ap, scale=s1ap)
                        # snake: r=(a/pi)z; dd=r-int(r); u=sin(pi dd);
                        # h = z + invb*u^2 (sin^2 is pi-periodic so the
                        # trunc-vs-round cast ambiguity is harmless)
                        r2 = p2.tile([P, TT2], F32, tag="r2")
                        nc.scalar.activation(r2, z2, AF.Identity, scale=apap)
                        ri = p2.tile([P, TT2], I32, tag="ri")
                        nc.vector.tensor_copy(ri, r2)
                        d2 = p2.tile([P, TT2], F32, tag="d2")
                        nc.vector.tensor_sub(d2, r2, ri)
                        u2 = p2.tile([P, TT2], BF16, tag="u2")
                        nc.scalar.activation(u2, d2, AF.Sin,
                                             scale=float(np.pi))
                        v2 = p2.tile([P, TT2], BF16, tag="v2")
                        nc.gpsimd.tensor_mul(v2, u2, u2)
                        # deinterleave h into he/ho (strided reads,
                        # contiguous writes)
                        hc0 = HPAD + jp * TT
                        nc.vector.scalar_tensor_tensor(
                            he[co][:, hc0:hc0 + TT],
                            v2[:, 0:TT2:2], ibap, z2[:, 0:TT2:2],
                            ALU.mult, ALU.add)
                        nc.vector.scalar_tensor_tensor(
                            ho[co][:, hc0:hc0 + TT],
                            v2[:, 1:TT2:2], ibap, z2[:, 1:TT2:2],
                            ALU.mult, ALU.add)
                    if jp in (1, 2):
                        # slab jp-1's inputs are complete (incl. the +1
                        # column from slab jp); keep 2 sets in flight
                        # (dh bufs=2), slabs 2,3 are emitted inside conv2
                        dh_ready[jp - 1] = emit_transforms(jp - 1)

                # ---- conv2 (winograd F(2,3), dilation 1) + residual ----
                b2aps = [pt[co][:, base + 4:base + 5] for co in range(NCH)]
                s2aps = [pt[co][:, base + 5:base + 6] for co in range(NCH)]
                for g in range(NG):
                    dh = dh_ready.pop(g)
                    for co in range(NCH):
                        # chains: A = m0+m2 (slots 0,2), B = m1 (slot 1),
                        #         D = -m2-m3 (slots 3,4)
                        psA = pps.tile([P, JW], F32, tag="ps")
                        psB = pps.tile([P, JW], F32, tag="ps")
                        psD = pps.tile([P, JW], F32, tag="ps")
                        for n, (pst, slot, st) in enumerate((
                                (psA, 0, 0), (psA, 2, 2),
                                (psB, 1, 1),
                                (psD, 3, 2), (psD, 4, 3))):
                            first = slot in (0, 1, 3)
                            last = slot in (2, 1, 4)
                            for ci in range(NCH):
                                nc.tensor.matmul(
                                    pst,
                                    w2t[ci][:, (co * NS2 + slot) * P:
                                            (co * NS2 + slot + 1) * P],
                                    dh[ci][:, st * JW:(st + 1) * JW],
                                    start=(first and ci == 0),
                                    stop=(last and ci == NCH - 1),
                                )
                        # walrus: a TensorTensor may read only ONE psum
                        # operand. Drain B via ScalarE with scale/bias
                        # folded (tB = s2*m1 + b2), then each output half
                        # is one stt reading a single psum bank:
                        #   t_even = s2*A + tB,  t_odd = s2*D + tB
                        # (tags shared with conv1-phase tiles, disjoint
                        # lifetime, to stay inside SBUF)
                        tBt = p2.tile([P, TT2], F32, tag="r2")
                        tB = tBt[:, 0:JW]
                        nc.scalar.activation(tB, psB, AF.Identity,
                                             bias=b2aps[co], scale=s2aps[co])
                        q01 = p2.tile([P, TT2], F32, tag="d2")
                        nc.vector.scalar_tensor_tensor(
                            q01[:, 0:JW], psA, s2aps[co], tB,
                            ALU.mult, ALU.add)
                        nc.vector.scalar_tensor_tensor(
                            q01[:, JW:2 * JW], psD, s2aps[co], tB,
                            ALU.mult, ALU.add)
                        col0 = PAD + g * TT2
                        if i < 2:
                            nc.vector.tensor_add(
                                xnxt[co][:, col0:col0 + TT2:2],
                                xcur[co][:, col0:col0 + TT2:2],
                                q01[:, 0:JW])
                            nc.vector.tensor_add(
                                xnxt[co][:, col0 + 1:col0 + TT2:2],
                                xcur[co][:, col0 + 1:col0 + TT2:2],
                                q01[:, JW:2 * JW])
                        else:
                            y2 = pz.tile([P, TT2], F32, tag="z2")
                            nc.vector.tensor_add(
                                y2[:, 0:TT2:2],
                                xcur[co][:, col0:col0 + TT2:2],
                                q01[:, 0:JW])
                            nc.vector.tensor_add(
                                y2[:, 1:TT2:2],
                                xcur[co][:, col0 + 1:col0 + TT2:2],
                                q01[:, JW:2 * JW])
                            nc.sync.dma_start(
                                out=y_d[co * P:(co + 1) * P,
                                        g * TT2:(g + 1) * TT2],
                                in_=y2)
                    if g + 2 < NG:
                        dh_ready[g + 2] = emit_transforms(g + 2)
                if i < 2:
                    wcur = wnext
    _split_sync_waits(nc)
    return nc


_NC = None


def _get_nc():
    global _NC
    if _NC is None:
        _NC = build_nc()
    return _NC


def _host_params(w1, b1, alpha, beta, w2, b2):
    """Ternarize weights and fold snake/scale params, matching the
    reference's jax-on-CPU float32 numerics."""
    import jax
    import jax.numpy as jnp

    cpu = jax.devices("cpu")[0]

    wt = np.zeros((3, 2, NCH, P, NS2 * NCH * P), dtype=ml_dtypes.bfloat16)
    pp = np.zeros((NCH, P, NPARAM), dtype=np.float32)
    pi = np.float32(np.pi)

    with jax.default_device(cpu):
        for i in range(3):
            svals = []
            for conv, w in ((0, w1[i]), (1, w2[i])):
                s = jnp.mean(jnp.abs(w))
                tern = jnp.clip(jnp.round(w / (s + EPS_Q)), -1.0, 1.0)
                svals.append(np.float32(s))
                tern = np.asarray(tern, dtype=np.float32)  # [co, ci, k]
                if conv == 0:
                    slots = tern  # direct: 3 tap slots
                    ns = NS1
                else:
                    # winograd slots: w^0, w^1, w^2, -w^2, -w^3
                    t0, t1, t2_ = tern[:, :, 0], tern[:, :, 1], tern[:, :, 2]
                    gw1 = (t0 + t1 + t2_) * np.float32(0.5)
                    gw2 = (t0 - t1 + t2_) * np.float32(0.5)
                    slots = np.stack([t0, gw1, gw2, -gw2, -t2_], axis=2)
                    ns = NS2
                # [co, ci, s] -> [cich, ci_in, coch, s, co_in]
                t5 = slots.reshape(NCH, P, NCH, P, ns).transpose(2, 3, 0, 4, 1)
                wt[i, conv, :, :, 0:ns * NCH * P] = t5.reshape(
                    NCH, P, ns * NCH * P).astype(ml_dtypes.bfloat16)
            s1, s2 = svals
            a = np.asarray(jnp.exp(alpha[i]), dtype=np.float32)
            bsn = np.asarray(jnp.exp(beta[i]), dtype=np.float32)
            invb = np.asarray(
                jnp.float32(1.0) / (jnp.asarray(bsn) + jnp.float32(EPS_SNAKE)),
                dtype=np.float32)
            base = i * 6
            pp[:, :, base + 0] = b1[i].reshape(NCH, P)
            pp[:, :, base + 1] = s1
            pp[:, :, base + 2] = (a / pi).reshape(NCH, P)
            pp[:, :, base + 3] = invb.reshape(NCH, P)
            pp[:, :, base + 4] = b2[i].reshape(NCH, P)
            pp[:, :, base + 5] = s2
    return wt, pp


def kernel(x, w1, b1, alpha, beta, w2, b2):
    global LAST_EXEC_NS
    x = np.asarray(x, dtype=np.float32)
    w1 = np.asarray(w1, dtype=np.float32)
    b1 = np.asarray(b1, dtype=np.float32)
    alpha = np.asarray(alpha, dtype=np.float32)
    beta = np.asarray(beta, dtype=np.float32)
    w2 = np.asarray(w2, dtype=np.float32)
    b2 = np.asarray(b2, dtype=np.float32)

    wt, pp = _host_params(w1, b1, alpha, beta, w2, b2)
    nc = _get_nc()

    in_maps = [
        {"xb16": x[b].astype(ml_dtypes.bfloat16), "wt": wt, "pp": pp}
        for b in range(B)
    ]
    res = run_bass_kernel_spmd(
        nc, in_maps, core_ids=list(range(B)), trace=TRACE)
    LAST_EXEC_NS = res.exec_time_ns
    global LAST_RESULT
    LAST_RESULT = res

    out = np.stack([res.results[b]["y"] for b in range(B)], axis=0)
    return out.astype(np.float32)
